# revision 5
# baseline (speedup 1.0000x reference)
"""Trainium2 Bass kernel for nn_AdvancedSpikingChatModel.

Model: spike-encode embeddings -> 6 spiking-transformer blocks (LIF gates +
decaying linear-attention recurrence over T=16) -> LIF output head with
spike-count accumulation over V=32000 vocab.

Strategy (8 NeuronCores, SPMD, two launches):
  Launch 1 (blocks): data-parallel over the 256 folded (b,s) rows, 32/core.
    Features on partitions, (t, row) on the free dim; weights stationary.
  Launch 2 (head): vocab-parallel, 4096 padded cols/core, all 256 rows.

All matmuls fp32 (fp32r/bf16 flip LIF thresholds). LIF decay 0.5 folded into
weights (v' = 0.5 v + x@(0.5 W); s = v' >= 1; v = min(v',1) - s).
PSUM discipline: every matmul accumulates into a 1-bank tile, ACT copies it
to SBUF (adding bias where needed), bank releases. Static PSUM budget:
mm 4 banks + stats 2 + broadcast 2 = 8.
"""

import numpy as np

import concourse.mybir as mybir
import concourse.tile as tile
from concourse import bacc
from concourse.bass_utils import run_bass_kernel_spmd

F32 = mybir.dt.float32
OP = mybir.AluOpType
AF = mybir.ActivationFunctionType

B, S, D, T, L, F, V = 2, 128, 256, 16, 6, 1024, 32000
N = B * S            # 256 folded rows
NCORE = 8
R = N // NCORE       # 32 rows per core (launch 1)
TR = T * R           # 512
KC = D // 128        # 2 contraction chunks for D
FC = F // 128        # 8 chunks for F
VPAD = 32768
VSH = VPAD // NCORE  # 4096 vocab per core (launch 2)
VCH = VSH // 128     # 32 chunks of 128 vocab rows
TN = T * N           # 4096 free (launch 2)
EPS = 1e-5

# packed per-layer weight slab offsets (fp32 words per partition)
GATE_OFF = 0                      # 12 tiles of [128,128]: (g, hf, kc)
WO_OFF = GATE_OFF + 12 * 128      # 4 tiles: (hf, kc)
W1_OFF = WO_OFF + 4 * 128         # 16 tiles: (mf, kc)
W2_OFF = W1_OFF + 16 * 128        # 16 tiles: (mh, kc8)
B1_OFF = W2_OFF + 16 * 128        # [128, 8]
B2_OFF = B1_OFF + 8               # [128, 2]
LN_OFF = B2_OFF + 2               # [128, 8]: g1, b1, g2, b2
LW = LN_OFF + 8


def _sigmoid(x):
    return 1.0 / (1.0 + np.exp(-x))


def _encode_spikes(input_ids, token_embedding, pos_embedding, noise, unif):
    """Host-side rate coding; (0.7*rate + 0.3*temp > 0.5) == rate exactly."""
    emb = token_embedding[input_ids] + pos_embedding[None, :S]
    p = np.clip(_sigmoid(emb) * 0.8 + 0.1 + noise * 0.05, 0.0, 1.0)
    return (unif < p[None]).astype(np.float32)


def _mm_to_sbuf(nc, ps, wl, w_off, rhs_fn, nk, dst_ap, bias=0.0, name="mmb"):
    """Accumulating matmuls (nk K-chunks) into a 1-bank psum tile, then ACT
    copy (+bias) into dst_ap."""
    bank = ps.tile([128, 512], F32, tag="mm", name=name)
    for k in range(nk):
        nc.tensor.matmul(
            bank[:], wl[:, w_off + k * 128: w_off + (k + 1) * 128], rhs_fn(k),
            start=(k == 0), stop=(k == nk - 1),
        )
    nc.scalar.activation(dst_ap, bank[:], AF.Identity, bias=bias, scale=1.0)


def _lif_scan(nc, v_state, s_dst_fn, a_src_fn):
    for t in range(T):
        a_t = a_src_fn(t)
        s_t = s_dst_fn(t)
        nc.vector.scalar_tensor_tensor(
            out=v_state, in0=v_state, scalar=0.5, in1=a_t, op0=OP.mult, op1=OP.add)
        nc.vector.tensor_scalar(
            out=s_t, in0=v_state, scalar1=1.0, scalar2=None, op0=OP.is_ge)
        nc.vector.scalar_tensor_tensor(
            out=v_state, in0=v_state, scalar=1.0, in1=s_t, op0=OP.min, op1=OP.subtract)


def _layer_norm(nc, ps, sb, u, sq_buf, gamma_col, beta_col, out_aps,
                ones_col, ones_row):
    """LN over features (partitions x KC chunks). u: [128, KC, TR] sbuf tile."""
    for kc in range(KC):
        nc.scalar.activation(sq_buf[:, kc, :], u[:, kc, :], AF.Square)
    ps_m = ps.tile([1, TR], F32, tag="st", name="ps_m")
    ps_q = ps.tile([1, TR], F32, tag="st", name="ps_q")
    for kc in range(KC):
        nc.tensor.matmul(ps_m[:], ones_col[:], u[:, kc, :],
                         start=(kc == 0), stop=(kc == KC - 1))
    for kc in range(KC):
        nc.tensor.matmul(ps_q[:], ones_col[:], sq_buf[:, kc, :],
                         start=(kc == 0), stop=(kc == KC - 1))
    m_sb = sb.tile([1, TR], F32, name="m_sb")
    q_sb = sb.tile([1, TR], F32, name="q_sb")
    nc.scalar.mul(m_sb[:], ps_m[:], 1.0 / D)
    nc.scalar.mul(q_sb[:], ps_q[:], 1.0 / D)
    ve = sb.tile([1, TR], F32, name="ve")
    nc.vector.tensor_mul(out=ve[:], in0=m_sb[:], in1=m_sb[:])
    nc.vector.tensor_sub(out=ve[:], in0=q_sb[:], in1=ve[:])
    nc.vector.tensor_scalar_add(out=ve[:], in0=ve[:], scalar1=EPS)
    r0 = sb.tile([1, TR], F32, name="r0")
    nc.scalar.activation(r0[:], ve[:], AF.Sqrt)
    nc.vector.reciprocal(out=r0[:], in_=r0[:])
    t1 = sb.tile([1, TR], F32, name="t1")
    nc.vector.tensor_mul(out=t1[:], in0=r0[:], in1=r0[:])
    nc.vector.tensor_mul(out=t1[:], in0=t1[:], in1=ve[:])
    nc.vector.tensor_scalar(out=t1[:], in0=t1[:], scalar1=-0.5, scalar2=1.5,
                            op0=OP.mult, op1=OP.add)
    nc.vector.tensor_mul(out=r0[:], in0=r0[:], in1=t1[:])
    pb_m = ps.tile([128, TR], F32, tag="bc", name="pb_m")
    pb_r = ps.tile([128, TR], F32, tag="bc", name="pb_r")
    nc.tensor.matmul(pb_m[:], ones_row[:], m_sb[:], start=True, stop=True)
    nc.tensor.matmul(pb_r[:], ones_row[:], r0[:], start=True, stop=True)
    for kc in range(KC):
        o = out_aps[kc]
        nc.vector.tensor_sub(out=o, in0=u[:, kc, :], in1=pb_m[:])
        nc.vector.tensor_mul(out=o, in0=o, in1=pb_r[:])
        nc.vector.tensor_scalar(out=o, in0=o, scalar1=gamma_col(kc),
                                scalar2=beta_col(kc), op0=OP.mult, op1=OP.add)


def build_blocks():
    nc = bacc.Bacc("TRN2", target_bir_lowering=False)
    x0_d = nc.dram_tensor("x0", [128, KC, TR], F32, kind="ExternalInput")
    wb_d = nc.dram_tensor("wblk", [L, 128, LW], F32, kind="ExternalInput")
    h_d = nc.dram_tensor("h_out", [128, KC, TR], F32, kind="ExternalOutput")

    with tile.TileContext(nc) as tc:
        with tc.tile_pool(name="wp", bufs=2) as wp, \
             tc.tile_pool(name="sb", bufs=1) as sb, \
             tc.tile_pool(name="ps", bufs=1, space="PSUM") as ps:

            ones_col = sb.tile([128, 1], F32)
            ones_row = sb.tile([1, 128], F32)
            nc.vector.memset(ones_col[:], 1.0)
            nc.vector.memset(ones_row[:], 1.0)

            x_cur = sb.tile([128, KC, TR], F32)
            nc.sync.dma_start(x_cur[:], x0_d.ap()[:])

            ag_buf = sb.tile([128, 6, TR], F32)       # gate pre-acts
            s_buf = sb.tile([128, T, 6, R], F32)      # gate spikes
            rh_buf = sb.tile([128, T, KC, R], F32)    # r*h
            at_buf = sb.tile([128, KC, TR], F32)      # attn out
            u_buf = sb.tile([128, KC, TR], F32)
            sq_buf = sb.tile([128, KC, TR], F32)
            x1_buf = sb.tile([128, KC, TR], F32)
            a1_buf = sb.tile([128, FC, TR], F32)      # ffn1 pre-acts
            s1_buf = sb.tile([128, T, FC, R], F32)
            a2_buf = sb.tile([128, KC, TR], F32)
            s2_buf = sb.tile([128, T, KC, R], F32)
            vg = sb.tile([128, 6, R], F32)
            hst = sb.tile([128, KC, R], F32)
            kv = sb.tile([128, KC, R], F32)
            v1 = sb.tile([128, FC, R], F32)
            v2 = sb.tile([128, KC, R], F32)

            wl_tiles = [wp.tile([128, LW], F32, tag="wl", name=f"wl{i}")
                        for i in range(L)]
            for l in range(L):
                nc.sync.dma_start(wl_tiles[l][:], wb_d.ap()[l])

            for l in range(L):
                wl = wl_tiles[l]
                nc.vector.memset(vg[:], 0.0)
                nc.vector.memset(hst[:], 0.0)
                nc.vector.memset(v1[:], 0.0)
                nc.vector.memset(v2[:], 0.0)

                # gates -> ag_buf
                for g in range(3):
                    for hf in range(KC):
                        bank = g * KC + hf
                        _mm_to_sbuf(
                            nc, ps, wl, GATE_OFF + bank * KC * 128,
                            lambda kc: x_cur[:, kc, :], KC,
                            ag_buf[:, bank, :], name=f"g{bank}")

                # gate LIF + h recurrence
                for t in range(T):
                    a_t = ag_buf[:, :, t * R:(t + 1) * R]
                    s_t = s_buf[:, t]
                    nc.vector.scalar_tensor_tensor(
                        out=vg[:], in0=vg[:], scalar=0.5, in1=a_t,
                        op0=OP.mult, op1=OP.add)
                    nc.vector.tensor_scalar(
                        out=s_t, in0=vg[:], scalar1=1.0, scalar2=None, op0=OP.is_ge)
                    nc.vector.scalar_tensor_tensor(
                        out=vg[:], in0=vg[:], scalar=1.0, in1=s_t,
                        op0=OP.min, op1=OP.subtract)
                    nc.vector.tensor_mul(out=kv[:], in0=s_buf[:, t, 2:4, :],
                                         in1=s_buf[:, t, 4:6, :])
                    nc.vector.scalar_tensor_tensor(
                        out=hst[:], in0=hst[:], scalar=0.9, in1=kv[:],
                        op0=OP.mult, op1=OP.add)
                    nc.vector.tensor_mul(out=rh_buf[:, t], in0=s_buf[:, t, 0:2, :],
                                         in1=hst[:])

                # attn = rh @ Wo -> at_buf
                for hf in range(KC):
                    _mm_to_sbuf(
                        nc, ps, wl, WO_OFF + hf * KC * 128,
                        lambda kc: rh_buf[:, :, kc, :], KC,
                        at_buf[:, hf, :], name=f"wo{hf}")

                # LN1(x + attn) -> x1
                for kc in range(KC):
                    nc.vector.tensor_add(out=u_buf[:, kc, :], in0=x_cur[:, kc, :],
                                         in1=at_buf[:, kc, :])
                _layer_norm(
                    nc, ps, sb, u_buf, sq_buf,
                    lambda kc: wl[:, LN_OFF + kc:LN_OFF + kc + 1],
                    lambda kc: wl[:, LN_OFF + 2 + kc:LN_OFF + 2 + kc + 1],
                    [x1_buf[:, kc, :] for kc in range(KC)],
                    ones_col, ones_row)

                # FFN mm1 (+b1) -> a1_buf
                for mf in range(FC):
                    _mm_to_sbuf(
                        nc, ps, wl, W1_OFF + mf * KC * 128,
                        lambda kc: x1_buf[:, kc, :], KC,
                        a1_buf[:, mf, :],
                        bias=wl[:, B1_OFF + mf:B1_OFF + mf + 1], name=f"f{mf}")

                _lif_scan(nc, v1[:], lambda t: s1_buf[:, t],
                          lambda t: a1_buf[:, :, t * R:(t + 1) * R])

                # mm2 (+b2) -> a2_buf
                for mh in range(KC):
                    _mm_to_sbuf(
                        nc, ps, wl, W2_OFF + mh * FC * 128,
                        lambda kc8: s1_buf[:, :, kc8, :], FC,
                        a2_buf[:, mh, :],
                        bias=wl[:, B2_OFF + mh:B2_OFF + mh + 1], name=f"m2{mh}")

                _lif_scan(nc, v2[:], lambda t: s2_buf[:, t],
                          lambda t: a2_buf[:, :, t * R:(t + 1) * R])

                # LN2(x1 + s2) -> x_cur
                for kc in range(KC):
                    nc.vector.tensor_add(out=u_buf[:, kc, :], in0=x1_buf[:, kc, :],
                                         in1=s2_buf[:, :, kc, :])
                _layer_norm(
                    nc, ps, sb, u_buf, sq_buf,
                    lambda kc: wl[:, LN_OFF + 4 + kc:LN_OFF + 4 + kc + 1],
                    lambda kc: wl[:, LN_OFF + 6 + kc:LN_OFF + 6 + kc + 1],
                    [x_cur[:, kc, :] for kc in range(KC)],
                    ones_col, ones_row)

            nc.sync.dma_start(h_d.ap()[:], x_cur[:])
    nc.compile()
    return nc


def build_head():
    nc = bacc.Bacc("TRN2", target_bir_lowering=False)
    h_d = nc.dram_tensor("hT", [128, KC, TN], F32, kind="ExternalInput")
    w_d = nc.dram_tensor("woutT", [128, KC * VCH * 128], F32, kind="ExternalInput")
    b_d = nc.dram_tensor("boutp", [128, VCH], F32, kind="ExternalInput")
    o_d = nc.dram_tensor("out_sh", [VCH, 128, N], F32, kind="ExternalOutput")

    with tile.TileContext(nc) as tc:
        with tc.tile_pool(name="sb", bufs=1) as sb, \
             tc.tile_pool(name="ab", bufs=2) as ab, \
             tc.tile_pool(name="ob", bufs=2) as ob, \
             tc.tile_pool(name="ps", bufs=8, space="PSUM") as ps:

            hT = sb.tile([128, KC, TN], F32)
            wout = sb.tile([128, KC * VCH * 128], F32)
            boutp = sb.tile([128, VCH], F32)
            nc.sync.dma_start(hT[:], h_d.ap()[:])
            nc.sync.dma_start(wout[:], w_d.ap()[:])
            nc.sync.dma_start(boutp[:], b_d.ap()[:])

            for c in range(VCH):
                a_buf = ab.tile([128, TN], F32, tag="a", name=f"a{c}")
                for fb in range(TN // 512):
                    bank = ps.tile([128, 512], F32, tag="mm", name=f"b{c}_{fb}")
                    for kc in range(KC):
                        off = (kc * VCH + c) * 128
                        nc.tensor.matmul(
                            bank[:], wout[:, off:off + 128],
                            hT[:, kc, fb * 512:(fb + 1) * 512],
                            start=(kc == 0), stop=(kc == KC - 1))
                    nc.scalar.activation(
                        a_buf[:, fb * 512:(fb + 1) * 512], bank[:],
                        AF.Identity, bias=boutp[:, c:c + 1], scale=1.0)

                v = ob.tile([128, N], F32, tag="v", name=f"v{c}")
                acc = ob.tile([128, N], F32, tag="acc", name=f"acc{c}")
                s_t = ob.tile([128, N], F32, tag="s", name=f"s{c}")
                nc.vector.memset(v[:], 0.0)
                nc.vector.memset(acc[:], 0.0)
                for t in range(T):
                    a_t = a_buf[:, t * N:(t + 1) * N]
                    nc.vector.scalar_tensor_tensor(
                        out=v[:], in0=v[:], scalar=0.5, in1=a_t,
                        op0=OP.mult, op1=OP.add)
                    nc.vector.tensor_scalar(
                        out=s_t[:], in0=v[:], scalar1=1.0, scalar2=None, op0=OP.is_ge)
                    nc.vector.scalar_tensor_tensor(
                        out=v[:], in0=v[:], scalar=1.0, in1=s_t[:],
                        op0=OP.min, op1=OP.subtract)
                    nc.vector.tensor_add(out=acc[:], in0=acc[:], in1=s_t[:])
                nc.sync.dma_start(o_d.ap()[c], acc[:])
    nc.compile()
    return nc


_CACHE = {}
TRACE = False       # test.py sets True (needs antenv.axon_hooks shim)
LAST = {}           # stash of BassKernelResults for test.py introspection


def _run(nc, in_maps, key):
    import tempfile

    if TRACE:
        td = tempfile.mkdtemp(prefix=f"bkt_{key}_")
        res = run_bass_kernel_spmd(nc, in_maps, core_ids=list(range(NCORE)),
                                   trace=True, tmpdir=td)
        LAST[key] = (res, td)
        return res
    return run_bass_kernel_spmd(nc, in_maps, core_ids=list(range(NCORE)))


def _get_programs():
    if "blocks" not in _CACHE:
        _CACHE["blocks"] = build_blocks()
        _CACHE["head"] = build_head()
    return _CACHE["blocks"], _CACHE["head"]


def _pack_wblk(Wr, Wk, Wv, Wo, W1, b1, W2, b2, g1, be1, g2, be2):
    out = np.zeros((L, 128, LW), np.float32)
    for l in range(L):
        cols = []
        for Wg in (Wr, Wk, Wv):
            for hf in range(KC):
                for kc in range(KC):
                    cols.append(0.5 * Wg[l][kc * 128:(kc + 1) * 128,
                                            hf * 128:(hf + 1) * 128])
        for hf in range(KC):
            for kc in range(KC):
                cols.append(Wo[l][kc * 128:(kc + 1) * 128, hf * 128:(hf + 1) * 128])
        for mf in range(FC):
            for kc in range(KC):
                cols.append(0.5 * W1[l][kc * 128:(kc + 1) * 128,
                                        mf * 128:(mf + 1) * 128])
        for mh in range(KC):
            for kc8 in range(FC):
                cols.append(0.5 * W2[l][kc8 * 128:(kc8 + 1) * 128,
                                        mh * 128:(mh + 1) * 128])
        slab = np.concatenate(cols, axis=1)
        smalls = np.concatenate([
            0.5 * b1[l].reshape(FC, 128).T,
            0.5 * b2[l].reshape(KC, 128).T,
            g1[l].reshape(KC, 128).T, be1[l].reshape(KC, 128).T,
            g2[l].reshape(KC, 128).T, be2[l].reshape(KC, 128).T,
        ], axis=1)
        out[l] = np.concatenate([slab, smalls], axis=1)
    return np.ascontiguousarray(out)


def kernel(input_ids, token_embedding, pos_embedding, noise, unif,
           Wr, Wk, Wv, Wo, W1, b1, W2, b2, ln1_g, ln1_b, ln2_g, ln2_b,
           Wout, bout):
    input_ids = np.asarray(input_ids)
    f32 = lambda a: np.asarray(a, dtype=np.float32)
    token_embedding, pos_embedding, noise, unif = map(
        f32, (token_embedding, pos_embedding, noise, unif))
    Wr, Wk, Wv, Wo, W1, b1, W2, b2 = map(f32, (Wr, Wk, Wv, Wo, W1, b1, W2, b2))
    ln1_g, ln1_b, ln2_g, ln2_b, Wout, bout = map(
        f32, (ln1_g, ln1_b, ln2_g, ln2_b, Wout, bout))

    nc_blocks, nc_head = _get_programs()

    spikes = _encode_spikes(input_ids, token_embedding, pos_embedding, noise, unif)
    sp = spikes.reshape(T, NCORE, R, KC, 128)          # (t, core, r, kc, p)
    x0 = np.ascontiguousarray(sp.transpose(1, 4, 3, 0, 2)).reshape(NCORE, 128, KC, TR)
    wblk = _pack_wblk(Wr, Wk, Wv, Wo, W1, b1, W2, b2, ln1_g, ln1_b, ln2_g, ln2_b)
    in1 = [{"x0": x0[c], "wblk": wblk} for c in range(NCORE)]
    res1 = _run(nc_blocks, in1, "blocks")
    ho = np.stack([res1.results[c]["h_out"].reshape(128, KC, T, R)
                   for c in range(NCORE)])
    hT = np.ascontiguousarray(ho.transpose(1, 2, 3, 0, 4)).reshape(128, KC, TN)

    Wp = np.zeros((D, VPAD), np.float32)
    Wp[:, :V] = 0.5 * Wout
    bp = np.zeros((VPAD,), np.float32)
    bp[:V] = 0.5 * bout
    in2 = []
    for c in range(NCORE):
        wsh = Wp[:, c * VSH:(c + 1) * VSH].reshape(KC, 128, VCH, 128)
        wsh = np.ascontiguousarray(wsh.transpose(1, 0, 2, 3)).reshape(128, KC * VCH * 128)
        bsh = np.ascontiguousarray(bp[c * VSH:(c + 1) * VSH].reshape(VCH, 128).T)
        in2.append({"hT": hT, "woutT": wsh, "boutp": bsh})
    res2 = _run(nc_head, in2, "head")
    out_sh = np.stack([res2.results[c]["out_sh"] for c in range(NCORE)])
    out = out_sh.reshape(VPAD, N)[:V]
    out = np.ascontiguousarray(out.T).reshape(B, S, V).astype(np.float32)
    return out
